# revision 1
# baseline (speedup 1.0000x reference)
"""BoltzmannRouter Trainium2 kernel: 8-core data-parallel Bass implementation.

Full inputs: x (4, 4096, 2048) f32, gate_w (64, 2048) f32.
Output: routing weights (4, 4096, 64) f32 (softmax -> top-44 mask -> renorm).

Sharding: 16384 tokens split 2048/core across 8 NeuronCores; gate weight
replicated. Host pre-transposes each x shard to [D, tokens] so the device
DMA loads contraction-major tiles at full bandwidth, and pre-scales gate_w
by 1/TEMPERATURE (and 2^6 in the fp16 path).

Matmul precision modes (BOLTZ_MM_MODE):
  fp16x3 (default): x and w each split into fp16 high + 2^-12-scaled fp16 low
    parts; scores = 2^-6*(A + 2^-12*B) with A = xh@wh, B = xh@wl + xl@wh
    accumulated in separate PSUM banks. Dropped terms ~2^-22 relative --
    below fp32 PSUM accumulation noise -- at 3 cyc/row instead of fp32's 4.
  fp32: native fp32 matmul (2 half-rate passes per matmul).
"""

import os
import sys

sys.path.insert(0, "/opt/trn_rl_repo")

import numpy as np

D = 2048
E = 64
N_BOTTOM = 20  # 64 experts - 44 active
EPS = 1e-8
NEG_BIG = -1e30
TEMPERATURE = 2.718281828459045
N_CORES = 8
TPC = 2048  # tokens per core
GROUP = 512  # tokens per matmul group (one PSUM bank)

W_SCALE = 64.0  # 2^6: lifts gate_w into fp16-normal range
LO_SCALE = 4096.0  # 2^12: scale on the low fp16 split parts

_MODE = os.environ.get("BOLTZ_MM_MODE", "fp16x3")


def _build_nc():
    import concourse.bacc as bacc
    import concourse.mybir as mybir
    from concourse.masks import make_identity
    from concourse.tile import TileContext

    F32 = mybir.dt.float32
    F16 = mybir.dt.float16
    fp16 = _MODE == "fp16x3"
    mm_dt = F16 if fp16 else getattr(mybir.dt, _MODE, F32)
    kc_n = D // 128
    n_groups = TPC // GROUP
    n_sub = GROUP // 128
    # psum_t carries (-scores) scaled by W_SCALE in the fp16 path
    inv_s = 1.0 / W_SCALE if fp16 else 1.0

    lean_tail = os.environ.get("BOLTZ_LEAN_TAIL", "1") == "1"
    if lean_tail:
        # the stock Tile exit emits drain + barrier + sem-clear + barrier
        # (~8us); the kernel preamble already range-clears the semaphores at
        # the start of every execution, so drain + one barrier suffices
        def _lean_drain_and_barrier(self, tick_clock, wait_clock):
            from concourse.tile import ScopedClock

            drain_inst = self.nc.sync.drain()
            wait_clock.add_sem_waits(
                drain_inst.ins, ScopedClock({None: tick_clock.global_clock})
            )
            self.nc.all_engine_barrier()
            popped = self.nc._tile_sem_poison_stack.pop()
            assert popped is self._sem_poison
            self.sems.allocated()

        TileContext._drain_and_barrier = _lean_drain_and_barrier

    nc = bacc.Bacc(None, target_bir_lowering=False)
    if fp16:
        # xpk[d, g, 0, :] = xh tokens of group g, xpk[d, g, 1, :] = xl
        xpk_d = nc.declare_dram_parameter(
            "xpk", [D, (TPC // GROUP) * 2 * GROUP], F16, isOutput=False
        )
        whl_d = nc.declare_dram_parameter("whl", [D, 2 * E], F16, isOutput=False)
    else:
        xT = nc.declare_dram_parameter("xT", [D, TPC], mm_dt, isOutput=False)
        wT = nc.declare_dram_parameter("wT", [D, E], mm_dt, isOutput=False)
    out = nc.declare_dram_parameter("out", [TPC, E], F32, isOutput=True)

    with TileContext(nc) as tc:
        with (
            tc.tile_pool(name="const", bufs=1) as cpool,
            tc.tile_pool(name="xg", bufs=4) as xpool,
            tc.tile_pool(name="sneg", bufs=2) as spool,
            tc.tile_pool(name="og", bufs=4) as opool,
            tc.tile_pool(name="work", bufs=3) as wkpool,
            tc.tile_pool(name="small", bufs=8) as smpool,
            tc.tile_pool(name="ps_s", bufs=2 if fp16 else 2, space="PSUM") as ps_s_pool,
            tc.tile_pool(name="ps_b", bufs=2, space="PSUM") as ps_b_pool,
            tc.tile_pool(name="ps_t", bufs=4, space="PSUM") as ps_t_pool,
        ):
            ident = cpool.tile([E, E], F32)
            make_identity(nc, ident)
            if fp16:
                # -I/W_SCALE: transposing with a normal matmul by this matrix
                # descales and negates the scores in one shot
                identn = cpool.tile([E, E], F32)
                nc.gpsimd.memset(identn, 0.0)
                nc.gpsimd.affine_select(
                    out=identn,
                    in_=identn,
                    compare_op=mybir.AluOpType.not_equal,
                    fill=-1.0 / W_SCALE,
                    base=0,
                    pattern=[[-1, E]],
                    channel_multiplier=1,
                )

            if fp16:
                whl_sb = cpool.tile([128, kc_n, 2 * E], F16)
                nc.sync.dma_start(
                    out=whl_sb, in_=whl_d[:, :].rearrange("(kc p) e -> p kc e", p=128)
                )
            else:
                w_sb = cpool.tile([128, kc_n, E], mm_dt)
                nc.sync.dma_start(
                    out=w_sb, in_=wT[:, :].rearrange("(kc p) e -> p kc e", p=128)
                )

            og_tiles = []
            for g in range(n_groups):
                tok = slice(g * GROUP, (g + 1) * GROUP)
                if fp16:
                    # per-(group, chunk) tiles so PE pipelines at DMA-arrival
                    # granularity; dispatches spread over 3 queues (SP issue
                    # cost is ~620ns per DMA regardless of size)
                    xhs, xls = [], []
                    gcols = slice(g * 2 * GROUP, (g + 1) * 2 * GROUP)
                    for kc in range(kc_n):
                        row = slice(kc * 128, (kc + 1) * 128)
                        xk = xpool.tile([128, 2 * GROUP], F16, tag=f"x{kc}")
                        nc.sync.dma_start(out=xk, in_=xpk_d[row, gcols])
                        xhs.append(xk[:, :GROUP])
                        xls.append(xk[:, GROUP:])
                    # packed stationary [wh|wl]: one matmul against xh gives
                    # A=wh.T@xh (rows 0:64) and B1=wl.T@xh (rows 64:128); the
                    # second against xl gives B2=wh.T@xl (rows 0:64, the
                    # wl.T@xl block is a free byproduct, never read).
                    # The last parent group is processed in two half-width
                    # passes so the final selection chain drains sooner.
                    snegs = []
                    splits = (
                        [(0, GROUP)]
                        if g < n_groups - 1
                        else [(0, GROUP // 2), (GROUP // 2, GROUP // 2)]
                    )
                    for xoff, w in splits:
                        ps1 = ps_s_pool.tile([2 * E, w], F32, tag="ps_a")
                        ps2 = ps_b_pool.tile([2 * E, w], F32, tag="ps_b")
                        for kc in range(kc_n):
                            nc.tensor.matmul(
                                ps1,
                                lhsT=whl_sb[:, kc, :],
                                rhs=xhs[kc][:, xoff : xoff + w],
                                start=(kc == 0), stop=(kc == kc_n - 1),
                            )
                        for kc in range(kc_n):
                            nc.tensor.matmul(
                                ps2,
                                lhsT=whl_sb[:, kc, :],
                                rhs=xls[kc][:, xoff : xoff + w],
                                start=(kc == 0), stop=(kc == kc_n - 1),
                            )
                        # sneg = A + 2^-12 (B1 + B2) = W_SCALE * scores (sign
                        # and descale are folded into the transpose matrix)
                        b2_sb = spool.tile([E, w], F32, tag="b2_sb")
                        nc.scalar.copy(b2_sb, ps2[:E, :])
                        bs = spool.tile([E, w], F32, tag="bs")
                        nc.vector.tensor_add(bs, ps1[E:, :], b2_sb)
                        sneg = spool.tile([E, w], F32, tag="sneg")
                        nc.vector.scalar_tensor_tensor(
                            out=sneg,
                            in0=bs,
                            scalar=1.0 / LO_SCALE,
                            in1=ps1[:E, :],
                            op0=mybir.AluOpType.mult,
                            op1=mybir.AluOpType.add,
                        )
                        snegs.append((xoff, w, sneg))
                else:
                    xgs = []
                    for kc in range(kc_n):
                        xk = xpool.tile([128, GROUP], mm_dt, tag=f"xg{kc}")
                        nc.sync.dma_start(
                            out=xk, in_=xT[kc * 128 : (kc + 1) * 128, tok]
                        )
                        xgs.append(xk)
                    psum_s = ps_s_pool.tile([E, GROUP], F32, tag="ps_a")
                    for kc in range(kc_n):
                        nc.tensor.matmul(
                            psum_s, lhsT=w_sb[:, kc, :], rhs=xgs[kc],
                            start=(kc == 0), stop=(kc == kc_n - 1),
                        )
                    sneg = spool.tile([E, GROUP], F32, tag="sneg")
                    nc.scalar.mul(sneg, psum_s, -1.0)
                    snegs = [(0, GROUP, sneg)]

                og = opool.tile([128, n_sub, E], F32, tag="og")

                for xoff, w, sneg in snegs:
                  for s in range(w // 128):
                    si = xoff // 128 + s
                    # token-major negated scores [128 tok, 64 e] (x W_SCALE)
                    psum_t = ps_t_pool.tile([128, E], F32, tag="ps_t")
                    if fp16:
                        nc.tensor.matmul(
                            psum_t,
                            lhsT=sneg[:, s * 128 : (s + 1) * 128],
                            rhs=identn,
                        )
                    else:
                        nc.tensor.transpose(
                            psum_t, sneg[:, s * 128 : (s + 1) * 128], ident
                        )

                    # exp bias: -max(scores) = inv_s * min(psum_t)
                    mn = smpool.tile([128, 1], F32, tag="mn")
                    nc.vector.tensor_reduce(
                        mn, psum_t, axis=mybir.AxisListType.X, op=mybir.AluOpType.min
                    )
                    # u = exp(scores - max); S = sum(u)
                    u = wkpool.tile([128, E], F32, tag="u")
                    S = smpool.tile([128, 1], F32, tag="S")
                    nc.scalar.activation(
                        u,
                        psum_t,
                        mybir.ActivationFunctionType.Exp,
                        bias=mn,
                        scale=-1.0,
                        accum_out=S,
                    )

                    # threshold = 21st smallest score (negated domain: top-8
                    # of -scores are the smallest scores; 2x8 removed, then
                    # rank 17-24 -> index 4 = 21st)
                    y = wkpool.tile([128, E], F32, tag="y")
                    nc.vector.tensor_copy(y, psum_t)
                    r1 = smpool.tile([128, 8], F32, tag="r1")
                    nc.vector.max(r1, y)
                    nc.vector.match_replace(y, r1, y, NEG_BIG)
                    r2 = smpool.tile([128, 8], F32, tag="r2")
                    nc.vector.max(r2, y)
                    nc.vector.match_replace(y, r2, y, NEG_BIG)
                    r3 = smpool.tile([128, 8], F32, tag="r3")
                    nc.vector.max(r3, y)
                    thr = r3[:, (N_BOTTOM - 16) : (N_BOTTOM - 16 + 1)]

                    # wm = u * (-scores <= thr); ws = sum(wm)
                    wm = wkpool.tile([128, E], F32, tag="wm")
                    ws = smpool.tile([128, 1], F32, tag="ws")
                    nc.vector.scalar_tensor_tensor(
                        out=wm,
                        in0=psum_t,
                        scalar=thr,
                        in1=u,
                        op0=mybir.AluOpType.is_le,
                        op1=mybir.AluOpType.mult,
                        accum_out=ws,
                    )
                    # den = S*eps + ws; out = wm * (1/den)
                    den = smpool.tile([128, 1], F32, tag="den")
                    nc.vector.scalar_tensor_tensor(
                        out=den,
                        in0=S,
                        scalar=EPS,
                        in1=ws,
                        op0=mybir.AluOpType.mult,
                        op1=mybir.AluOpType.add,
                    )
                    rd = smpool.tile([128, 1], F32, tag="rd")
                    nc.vector.reciprocal(rd, den)
                    nc.vector.tensor_scalar_mul(og[:, si, :], wm, rd)

                og_tiles.append(og)

            # all output DMAs at the very end of the SP stream so no x
            # prefetch dispatch ever queues behind an output wait
            for g, og in enumerate(og_tiles):
                nc.sync.dma_start(
                    out=out[g * GROUP : (g + 1) * GROUP, :].rearrange(
                        "(s p) e -> p s e", p=128
                    ),
                    in_=og,
                )

    nc.finalize()
    return nc


_NC = None
LAST_EXEC_NS = None
LAST_RESULTS = None


def _get_nc():
    global _NC
    if _NC is None:
        _NC = _build_nc()
    return _NC


def _split_fp16(a, scale_hi=1.0):
    """a (f32) -> (hi fp16, lo fp16) with a*scale_hi ~= hi + lo/LO_SCALE."""
    s = (a.astype(np.float32) * np.float32(scale_hi)).astype(np.float32)
    hi = s.astype(np.float16)
    lo = ((s - hi.astype(np.float32)) * np.float32(LO_SCALE)).astype(np.float16)
    return hi, lo


def kernel(x, gate_w, trace=False):
    global LAST_EXEC_NS, LAST_RESULTS
    from concourse.bass_utils import run_bass_kernel_spmd

    x = np.asarray(x)
    gate_w = np.asarray(gate_w)
    Btot = x.shape[0] * x.shape[1]
    x2 = np.ascontiguousarray(x.reshape(Btot, D).astype(np.float32, copy=False))
    wt = np.ascontiguousarray(
        gate_w.astype(np.float32, copy=False).T / np.float32(TEMPERATURE)
    )

    nc = _get_nc()
    in_maps = []
    if _MODE == "fp16x3":
        wh, wl = _split_fp16(wt, W_SCALE)
        whl = np.ascontiguousarray(np.concatenate([wh, wl], axis=1))
        ng = TPC // GROUP
        for i in range(N_CORES):
            shard = np.ascontiguousarray(x2[i * TPC : (i + 1) * TPC].T)
            xh, xl = _split_fp16(shard)
            xpk = np.empty((D, ng, 2, GROUP), np.float16)
            xpk[:, :, 0, :] = xh.reshape(D, ng, GROUP)
            xpk[:, :, 1, :] = xl.reshape(D, ng, GROUP)
            in_maps.append({"xpk": xpk.reshape(D, ng * 2 * GROUP), "whl": whl})
    else:
        for i in range(N_CORES):
            shard = np.ascontiguousarray(x2[i * TPC : (i + 1) * TPC].T)
            in_maps.append({"xT": shard, "wT": wt})

    kwargs = {}
    if trace:
        try:
            import antenv.axon_hooks  # noqa: F401  (shimmed by test harness)

            kwargs["trace"] = True
        except ImportError:
            pass
    res = run_bass_kernel_spmd(nc, in_maps, core_ids=list(range(N_CORES)), **kwargs)
    LAST_EXEC_NS = res.exec_time_ns
    LAST_RESULTS = res
    out = np.concatenate([res.results[i]["out"] for i in range(N_CORES)], axis=0)
    return out.reshape(x.shape[0], x.shape[1], E)



# revision 4
# speedup vs baseline: 1.4138x; 1.4138x over previous
"""BoltzmannRouter Trainium2 kernel: 8-core data-parallel Bass implementation.

Full inputs: x (4, 4096, 2048) f32, gate_w (64, 2048) f32.
Output: routing weights (4, 4096, 64) f32 (softmax -> top-44 mask -> renorm).

Sharding: 16384 tokens split 2048/core across 8 NeuronCores; gate weight
replicated. Host pre-transposes each x shard to [D, tokens] fp16 and
pre-scales gate_w by W_SCALE/TEMPERATURE, split into fp16 high+low parts.

v2 design notes (per core):
  - x is shipped fp16-high only (8.4MB vs 16.8MB): x rounding adds ~1.4e-4
    score noise, well under the 2e-2 rel-err gate even counting top-k
    boundary swaps. w keeps the hi+lo fp16 split (free: packed stationary).
  - scores matmul: stationary whl [128, 2E], moving xh [128, 512] fp16,
    16 k-chunks accumulate into one PSUM bank per 512-token group.
  - The -1/W descale rides the PSUM->SBUF copy (scalar engine); the wl
    correction (STT) runs on GPSIMD; transpose to token-major is a pure
    tensor-engine transpose (2 cyc/row fp32) with a plain identity.
  - Softmax skips the max-subtraction (|scores| < ~4, exp is safe in fp32,
    and the renorm makes the shift mathematically irrelevant) and drops the
    +eps term (ws >= 44*e^-4 >> eps*S contribution ~1e-6 relative).
  - Per-subtile DVE chain is only max8 x3 + match_replace x2 + select-STT +
    reciprocal; exp runs on scalar, final scale on GPSIMD, output in fp16.
"""

import os
import sys

sys.path.insert(0, "/opt/trn_rl_repo")

import numpy as np

D = 2048
E = 64
N_BOTTOM = 20  # 64 experts - 44 active
NEG_BIG = -1e30
TEMPERATURE = 2.718281828459045
N_CORES = 8
TPC = 2048  # tokens per core
GROUP = 512  # tokens per matmul group (one PSUM bank)
KC_N = D // 128  # 16 contraction chunks
QN = 4  # kc chunks per x DMA (4 DMAs per group)

W_SCALE = 64.0  # 2^6: lifts the wl split into fp16-normal range
LO_SCALE = 4096.0  # 2^12: scale on the low fp16 split part

_OG_ENGINE = os.environ.get("BOLTZ_OG_ENGINE", "gpsimd")
# GPSIMD cannot access PSUM (verifier NCC_IBIR), so the combine (which reads
# the wl half straight from PSUM) must run on the vector engine
_COMBINE_ENGINE = os.environ.get("BOLTZ_COMBINE_ENGINE", "vector")
_MR_OOP = os.environ.get("BOLTZ_MR_OOP", "1") == "1"
_SPLIT_LAST = os.environ.get("BOLTZ_SPLIT_LAST", "1") == "1"


def _build_nc():
    import concourse.bacc as bacc
    import concourse.mybir as mybir
    from concourse.masks import make_identity
    from concourse.tile import TileContext

    F32 = mybir.dt.float32
    F16 = mybir.dt.float16
    n_groups = TPC // GROUP

    if os.environ.get("BOLTZ_LEAN_TAIL", "1") == "1":
        # the stock Tile exit emits drain + barrier + sem-clear + barrier
        # (~8us); the kernel preamble already range-clears the semaphores at
        # the start of every execution, so drain + one barrier suffices
        def _lean_drain_and_barrier(self, tick_clock, wait_clock):
            from concourse.tile import ScopedClock

            drain_inst = self.nc.sync.drain()
            wait_clock.add_sem_waits(
                drain_inst.ins, ScopedClock({None: tick_clock.global_clock})
            )
            self.nc.all_engine_barrier()
            popped = self.nc._tile_sem_poison_stack.pop()
            assert popped is self._sem_poison
            self.sems.allocated()

        TileContext._drain_and_barrier = _lean_drain_and_barrier

    nc = bacc.Bacc(None, target_bir_lowering=False)
    xT_d = nc.declare_dram_parameter("xT", [D, TPC], F16, isOutput=False)
    whl_d = nc.declare_dram_parameter("whl", [D, 2 * E], F16, isOutput=False)
    out_d = nc.declare_dram_parameter("out", [TPC, E], F16, isOutput=True)

    with TileContext(nc) as tc:
        with (
            tc.tile_pool(name="const", bufs=1) as cpool,
            tc.tile_pool(name="xg", bufs=n_groups) as xpool,
            tc.tile_pool(name="sab", bufs=2) as sabpool,
            tc.tile_pool(name="sneg", bufs=2) as snpool,
            tc.tile_pool(name="og", bufs=n_groups) as opool,
            tc.tile_pool(name="work", bufs=3) as wkpool,
            tc.tile_pool(name="small", bufs=8) as smpool,
            tc.tile_pool(name="ps_s", bufs=2, space="PSUM") as pspool,
            tc.tile_pool(name="ps_t", bufs=4, space="PSUM") as ps_t_pool,
        ):
            ident = cpool.tile([E, E], F32)
            make_identity(nc, ident)

            whl_sb = cpool.tile([128, KC_N, 2 * E], F16)
            nc.sync.dma_start(
                out=whl_sb, in_=whl_d[:, :].rearrange("(kc p) e -> p kc e", p=128)
            )

            # all x DMAs up front: no dependencies, SP issues them
            # back-to-back so the transfer stream never starves
            xgs = []  # xgs[g][q] = [128, QN, GROUP] fp16
            for g in range(n_groups):
                tiles = []
                for q in range(KC_N // QN):
                    xq = xpool.tile([128, QN, GROUP], F16, tag=f"xq{q}")
                    nc.sync.dma_start(
                        out=xq,
                        in_=xT_d[
                            q * QN * 128 : (q + 1) * QN * 128,
                            g * GROUP : (g + 1) * GROUP,
                        ].rearrange("(c p) t -> p c t", p=128),
                    )
                    tiles.append(xq)
                xgs.append(tiles)

            combine_eng = nc.gpsimd if _COMBINE_ENGINE == "gpsimd" else nc.vector
            og_eng = nc.gpsimd if _OG_ENGINE == "gpsimd" else nc.vector

            for g in range(n_groups):
                splits = (
                    [(0, GROUP // 2), (GROUP // 2, GROUP // 2)]
                    if (_SPLIT_LAST and g == n_groups - 1)
                    else [(0, GROUP)]
                )
                og = opool.tile([128, GROUP // 128, E], F16, tag="og")
                for xoff, w in splits:
                    ps = pspool.tile([128, w], F32, tag="ps")
                    for kc in range(KC_N):
                        nc.tensor.matmul(
                            ps,
                            lhsT=whl_sb[:, kc, :],
                            rhs=xgs[g][kc // QN][:, kc % QN, xoff : xoff + w],
                            start=(kc == 0),
                            stop=(kc == KC_N - 1),
                        )
                    # sa = -(1/W) * A; descale+negate rides the PSUM->SBUF copy
                    sa = sabpool.tile([E, w], F32, tag="sa")
                    nc.scalar.mul(sa, ps[:E, :], -1.0 / W_SCALE)
                    # sneg = -scores = sa - B/(W*4096); B read straight from
                    # PSUM (one PSUM operand is legal; two SBUF operands with
                    # different base partitions are not)
                    sneg = snpool.tile([E, w], F32, tag="sneg")
                    combine_eng.scalar_tensor_tensor(
                        out=sneg,
                        in0=ps[E:, :],
                        scalar=-1.0 / (W_SCALE * LO_SCALE),
                        in1=sa,
                        op0=mybir.AluOpType.mult,
                        op1=mybir.AluOpType.add,
                    )

                    for s in range(w // 128):
                        si = xoff // 128 + s
                        # token-major negated scores [128 tok, 64 e]
                        psum_t = ps_t_pool.tile([128, E], F32, tag="ps_t")
                        nc.tensor.transpose(
                            psum_t, sneg[:, s * 128 : (s + 1) * 128], ident
                        )
                        # u = exp(scores) (no max-sub needed: |scores| < ~4)
                        u = wkpool.tile([128, E], F32, tag="u")
                        nc.scalar.activation(
                            u, psum_t, mybir.ActivationFunctionType.Exp, scale=-1.0
                        )
                        # SBUF copy of -scores for the DVE selection chain
                        s_sb = wkpool.tile([128, E], F32, tag="s_sb")
                        nc.scalar.copy(s_sb, psum_t)

                        # threshold = 21st smallest score = 21st largest of
                        # -scores: top-8 rounds with match_replace, then
                        # rank 17-24 -> index 4
                        r1 = smpool.tile([128, 8], F32, tag="r1")
                        nc.vector.max(r1, s_sb)
                        y = wkpool.tile([128, E], F32, tag="y")
                        if _MR_OOP:
                            nc.vector.match_replace(y, r1, s_sb, NEG_BIG)
                        else:
                            nc.vector.tensor_copy(y, s_sb)
                            nc.vector.match_replace(y, r1, y, NEG_BIG)
                        r2 = smpool.tile([128, 8], F32, tag="r2")
                        nc.vector.max(r2, y)
                        nc.vector.match_replace(y, r2, y, NEG_BIG)
                        r3 = smpool.tile([128, 8], F32, tag="r3")
                        nc.vector.max(r3, y)
                        thr = r3[:, (N_BOTTOM - 16) : (N_BOTTOM - 16 + 1)]

                        # wm = u * (-scores <= thr); ws = sum(wm)
                        wm = wkpool.tile([128, E], F32, tag="wm")
                        ws = smpool.tile([128, 1], F32, tag="ws")
                        nc.vector.scalar_tensor_tensor(
                            out=wm,
                            in0=s_sb,
                            scalar=thr,
                            in1=u,
                            op0=mybir.AluOpType.is_le,
                            op1=mybir.AluOpType.mult,
                            accum_out=ws,
                        )
                        # out = wm / ws (the +eps term is ~1e-6 relative:
                        # ws >= 44*e^-4; dropped)
                        rd = smpool.tile([128, 1], F32, tag="rd")
                        nc.vector.reciprocal(rd, ws)
                        og_eng.tensor_scalar_mul(og[:, si, :], wm, rd)

                # inline output DMA: all x DMAs are already issued, so this
                # never delays a prefetch; draining per group shortens the tail
                nc.sync.dma_start(
                    out=out_d[g * GROUP : (g + 1) * GROUP, :].rearrange(
                        "(s p) e -> p s e", p=128
                    ),
                    in_=og,
                )

    nc.finalize()
    return nc


_NC = None
LAST_EXEC_NS = None
LAST_RESULTS = None


def _get_nc():
    global _NC
    if _NC is None:
        _NC = _build_nc()
    return _NC


def kernel(x, gate_w, trace=False):
    global LAST_EXEC_NS, LAST_RESULTS
    from concourse.bass_utils import run_bass_kernel_spmd

    x = np.asarray(x)
    gate_w = np.asarray(gate_w)
    Btot = x.shape[0] * x.shape[1]
    x2 = x.reshape(Btot, D).astype(np.float32, copy=False)
    # w scaled by W_SCALE/T, split fp16 hi + (residual*LO_SCALE) lo, packed
    wt = gate_w.astype(np.float32).T * np.float32(W_SCALE / TEMPERATURE)
    wh = wt.astype(np.float16)
    wl = ((wt - wh.astype(np.float32)) * np.float32(LO_SCALE)).astype(np.float16)
    whl = np.ascontiguousarray(np.concatenate([wh, wl], axis=1))

    nc = _get_nc()
    in_maps = []
    for i in range(N_CORES):
        shard = np.ascontiguousarray(x2[i * TPC : (i + 1) * TPC].T.astype(np.float16))
        in_maps.append({"xT": shard, "whl": whl})

    kwargs = {}
    if trace:
        try:
            import antenv.axon_hooks  # noqa: F401  (registered by tracehook)

            kwargs["trace"] = True
        except ImportError:
            pass
    res = run_bass_kernel_spmd(nc, in_maps, core_ids=list(range(N_CORES)), **kwargs)
    LAST_EXEC_NS = res.exec_time_ns
    LAST_RESULTS = res
    out = np.concatenate([res.results[i]["out"] for i in range(N_CORES)], axis=0)
    return out.reshape(x.shape[0], x.shape[1], E).astype(np.float32)


# revision 7
# speedup vs baseline: 1.4334x; 1.0138x over previous
"""BoltzmannRouter Trainium2 kernel: 8-core data-parallel Bass implementation.

Full inputs: x (4, 4096, 2048) f32, gate_w (64, 2048) f32.
Output: routing weights (4, 4096, 64) f32 (softmax -> top-44 mask -> renorm).

Sharding: 16384 tokens split 2048/core across 8 NeuronCores; gate weight
replicated. Host pre-transposes each x shard to [D, tokens] fp16 and
pre-scales gate_w by W_SCALE/TEMPERATURE, split into fp16 high+low parts.

v2 design notes (per core):
  - x is shipped fp16-high only (8.4MB vs 16.8MB): x rounding adds ~1.4e-4
    score noise, well under the 2e-2 rel-err gate even counting top-k
    boundary swaps. w keeps the hi+lo fp16 split (free: packed stationary).
  - scores matmul: stationary whl [128, 2E], moving xh [128, 512] fp16,
    16 k-chunks accumulate into one PSUM bank per 512-token group.
  - The -1/W descale rides the PSUM->SBUF copy (scalar engine); the wl
    correction (STT) runs on GPSIMD; transpose to token-major is a pure
    tensor-engine transpose (2 cyc/row fp32) with a plain identity.
  - Softmax skips the max-subtraction (|scores| < ~4, exp is safe in fp32,
    and the renorm makes the shift mathematically irrelevant) and drops the
    +eps term (ws >= 44*e^-4 >> eps*S contribution ~1e-6 relative).
  - Per-subtile DVE chain is only max8 x3 + match_replace x2 + select-STT +
    reciprocal; exp runs on scalar, final scale on GPSIMD, output in fp16.
"""

import os
import sys

sys.path.insert(0, "/opt/trn_rl_repo")

import numpy as np

D = 2048
E = 64
N_BOTTOM = 20  # 64 experts - 44 active
NEG_BIG = -1e30
TEMPERATURE = 2.718281828459045
N_CORES = 8
TPC = 2048  # tokens per core
GROUP = 512  # tokens per matmul group (one PSUM bank)
KC_N = D // 128  # 16 contraction chunks
QN = 4  # kc chunks per x DMA (4 DMAs per group)

W_SCALE = 64.0  # 2^6: lifts the wl split into fp16-normal range
LO_SCALE = 4096.0  # 2^12: scale on the low fp16 split part

# scalar: og = wm * rd rides an activation-Copy with per-partition scale.
# gpsimd measured 1171ns/op for this (vs ~350 scalar, ~250 DVE) — never use it.
_OG_ENGINE = os.environ.get("BOLTZ_OG_ENGINE", "scalar")
# GPSIMD cannot access PSUM (verifier NCC_IBIR), so the combine (which reads
# the wl half straight from PSUM) must run on the vector engine
_COMBINE_ENGINE = os.environ.get("BOLTZ_COMBINE_ENGINE", "vector")
_MR_OOP = os.environ.get("BOLTZ_MR_OOP", "1") == "1"
_SPLIT_LAST = os.environ.get("BOLTZ_SPLIT_LAST", "1") == "1"


def _build_nc():
    import concourse.bacc as bacc
    import concourse.mybir as mybir
    from concourse.masks import make_identity
    from concourse.tile import TileContext

    F32 = mybir.dt.float32
    F16 = mybir.dt.float16
    n_groups = TPC // GROUP

    if os.environ.get("BOLTZ_LEAN_TAIL", "1") == "1":
        # the stock Tile exit emits drain + barrier + sem-clear + barrier
        # (~8us); the kernel preamble already range-clears the semaphores at
        # the start of every execution, so drain + one barrier suffices
        def _lean_drain_and_barrier(self, tick_clock, wait_clock):
            from concourse.tile import ScopedClock

            drain_inst = self.nc.sync.drain()
            wait_clock.add_sem_waits(
                drain_inst.ins, ScopedClock({None: tick_clock.global_clock})
            )
            self.nc.all_engine_barrier()
            popped = self.nc._tile_sem_poison_stack.pop()
            assert popped is self._sem_poison
            self.sems.allocated()

        TileContext._drain_and_barrier = _lean_drain_and_barrier

    nc = bacc.Bacc(None, target_bir_lowering=False)
    xT_d = nc.declare_dram_parameter("xT", [D, TPC], F16, isOutput=False)
    whl_d = nc.declare_dram_parameter("whl", [D, 2 * E], F16, isOutput=False)
    out_d = nc.declare_dram_parameter("out", [TPC, E], F16, isOutput=True)

    with TileContext(nc) as tc:
        with (
            tc.tile_pool(name="const", bufs=1) as cpool,
            tc.tile_pool(name="xg", bufs=n_groups) as xpool,
            tc.tile_pool(name="sab", bufs=2) as sabpool,
            tc.tile_pool(name="sneg", bufs=2) as snpool,
            tc.tile_pool(name="og", bufs=n_groups) as opool,
            tc.tile_pool(name="work", bufs=3) as wkpool,
            tc.tile_pool(name="small", bufs=8) as smpool,
            tc.tile_pool(name="ps_s", bufs=2, space="PSUM") as pspool,
            tc.tile_pool(name="ps_t", bufs=4, space="PSUM") as ps_t_pool,
        ):
            ident = cpool.tile([E, E], F32)
            make_identity(nc, ident)

            whl_sb = cpool.tile([128, KC_N, 2 * E], F16)
            nc.sync.dma_start(
                out=whl_sb, in_=whl_d[:, :].rearrange("(kc p) e -> p kc e", p=128)
            )

            # all x DMAs up front: no dependencies, SP issues them
            # back-to-back so the transfer stream never starves
            xgs = []  # xgs[g][q] = [128, QN, GROUP] fp16
            for g in range(n_groups):
                tiles = []
                for q in range(KC_N // QN):
                    xq = xpool.tile([128, QN, GROUP], F16, tag=f"xq{q}")
                    nc.sync.dma_start(
                        out=xq,
                        in_=xT_d[
                            q * QN * 128 : (q + 1) * QN * 128,
                            g * GROUP : (g + 1) * GROUP,
                        ].rearrange("(c p) t -> p c t", p=128),
                    )
                    tiles.append(xq)
                xgs.append(tiles)

            combine_eng = nc.gpsimd if _COMBINE_ENGINE == "gpsimd" else nc.vector

            for g in range(n_groups):
                splits = (
                    [(0, GROUP // 2), (GROUP // 2, GROUP // 2)]
                    if (_SPLIT_LAST and g == n_groups - 1)
                    else [(0, GROUP)]
                )
                og = opool.tile([128, GROUP // 128, E], F16, tag="og")
                for xoff, w in splits:
                    ps = pspool.tile([128, w], F32, tag="ps")
                    for kc in range(KC_N):
                        nc.tensor.matmul(
                            ps,
                            lhsT=whl_sb[:, kc, :],
                            rhs=xgs[g][kc // QN][:, kc % QN, xoff : xoff + w],
                            start=(kc == 0),
                            stop=(kc == KC_N - 1),
                        )
                    # sa = -(1/W) * A; descale+negate rides the PSUM->SBUF copy
                    sa = sabpool.tile([E, w], F32, tag="sa")
                    nc.scalar.mul(sa, ps[:E, :], -1.0 / W_SCALE)
                    # sneg = -scores = sa - B/(W*4096); B read straight from
                    # PSUM (one PSUM operand is legal; two SBUF operands with
                    # different base partitions are not)
                    sneg = snpool.tile([E, w], F32, tag="sneg")
                    combine_eng.scalar_tensor_tensor(
                        out=sneg,
                        in0=ps[E:, :],
                        scalar=-1.0 / (W_SCALE * LO_SCALE),
                        in1=sa,
                        op0=mybir.AluOpType.mult,
                        op1=mybir.AluOpType.add,
                    )

                    for s in range(w // 128):
                        si = xoff // 128 + s
                        # token-major negated scores [128 tok, 64 e]
                        psum_t = ps_t_pool.tile([128, E], F32, tag="ps_t")
                        nc.tensor.transpose(
                            psum_t, sneg[:, s * 128 : (s + 1) * 128], ident
                        )
                        # u = exp(scores) (no max-sub needed: |scores| < ~4)
                        u = wkpool.tile([128, E], F32, tag="u")
                        nc.scalar.activation(
                            u, psum_t, mybir.ActivationFunctionType.Exp, scale=-1.0
                        )
                        # SBUF copy of -scores for the DVE selection chain
                        s_sb = wkpool.tile([128, E], F32, tag="s_sb")
                        nc.scalar.copy(s_sb, psum_t)

                        # threshold = 21st smallest score = 21st largest of
                        # -scores: top-8 rounds with match_replace, then
                        # rank 17-24 -> index 4
                        r1 = smpool.tile([128, 8], F32, tag="r1")
                        nc.vector.max(r1, s_sb)
                        y = wkpool.tile([128, E], F32, tag="y")
                        if _MR_OOP:
                            nc.vector.match_replace(y, r1, s_sb, NEG_BIG)
                        else:
                            nc.vector.tensor_copy(y, s_sb)
                            nc.vector.match_replace(y, r1, y, NEG_BIG)
                        r2 = smpool.tile([128, 8], F32, tag="r2")
                        nc.vector.max(r2, y)
                        nc.vector.match_replace(y, r2, y, NEG_BIG)
                        r3 = smpool.tile([128, 8], F32, tag="r3")
                        nc.vector.max(r3, y)
                        thr = r3[:, (N_BOTTOM - 16) : (N_BOTTOM - 16 + 1)]

                        # wm = u * (-scores <= thr); ws = sum(wm)
                        wm = wkpool.tile([128, E], F32, tag="wm")
                        ws = smpool.tile([128, 1], F32, tag="ws")
                        nc.vector.scalar_tensor_tensor(
                            out=wm,
                            in0=s_sb,
                            scalar=thr,
                            in1=u,
                            op0=mybir.AluOpType.is_le,
                            op1=mybir.AluOpType.mult,
                            accum_out=ws,
                        )
                        # out = wm / ws (the +eps term is ~1e-6 relative:
                        # ws >= 44*e^-4; dropped)
                        rd = smpool.tile([128, 1], F32, tag="rd")
                        nc.vector.reciprocal(rd, ws)
                        if _OG_ENGINE == "scalar":
                            nc.scalar.mul(og[:, si, :], wm, rd)
                        else:
                            nc.vector.tensor_scalar_mul(og[:, si, :], wm, rd)

                    # inline output DMA per split: all x DMAs are already
                    # issued so this never delays a prefetch, and draining
                    # per split shortens the final-group tail
                    nc.sync.dma_start(
                        out=out_d[
                            g * GROUP + xoff : g * GROUP + xoff + w, :
                        ].rearrange("(s p) e -> p s e", p=128),
                        in_=og[:, xoff // 128 : (xoff + w) // 128, :],
                    )

    nc.finalize()
    return nc


_NC = None
LAST_EXEC_NS = None
LAST_RESULTS = None


def _get_nc():
    global _NC
    if _NC is None:
        _NC = _build_nc()
    return _NC


def kernel(x, gate_w, trace=False):
    global LAST_EXEC_NS, LAST_RESULTS
    from concourse.bass_utils import run_bass_kernel_spmd

    x = np.asarray(x)
    gate_w = np.asarray(gate_w)
    Btot = x.shape[0] * x.shape[1]
    x2 = x.reshape(Btot, D).astype(np.float32, copy=False)
    # w scaled by W_SCALE/T, split fp16 hi + (residual*LO_SCALE) lo, packed
    wt = gate_w.astype(np.float32).T * np.float32(W_SCALE / TEMPERATURE)
    wh = wt.astype(np.float16)
    wl = ((wt - wh.astype(np.float32)) * np.float32(LO_SCALE)).astype(np.float16)
    whl = np.ascontiguousarray(np.concatenate([wh, wl], axis=1))

    nc = _get_nc()
    in_maps = []
    for i in range(N_CORES):
        shard = np.ascontiguousarray(x2[i * TPC : (i + 1) * TPC].T.astype(np.float16))
        in_maps.append({"xT": shard, "whl": whl})

    kwargs = {}
    if trace:
        try:
            import antenv.axon_hooks  # noqa: F401  (registered by tracehook)

            kwargs["trace"] = True
        except ImportError:
            pass
    res = run_bass_kernel_spmd(nc, in_maps, core_ids=list(range(N_CORES)), **kwargs)
    LAST_EXEC_NS = res.exec_time_ns
    LAST_RESULTS = res
    out = np.concatenate([res.results[i]["out"] for i in range(N_CORES)], axis=0)
    return out.reshape(x.shape[0], x.shape[1], E).astype(np.float32)


# revision 8
# speedup vs baseline: 1.4558x; 1.0156x over previous
"""BoltzmannRouter Trainium2 kernel: 8-core data-parallel Bass implementation.

Full inputs: x (4, 4096, 2048) f32, gate_w (64, 2048) f32.
Output: routing weights (4, 4096, 64) f32 (softmax -> top-44 mask -> renorm).

Sharding: 16384 tokens split 2048/core across 8 NeuronCores; gate weight
replicated. Host pre-transposes each x shard to [D, tokens] fp16 and
pre-negates/scales gate_w to -gate_w.T/TEMPERATURE in fp16.

v4 design notes (per core):
  - x and w both ship fp16 (8.6MB/core vs 17.3 for fp32): the rounding adds
    ~2e-4 score noise -> ~0.3% of tokens swap a boundary expert, ~5e-3 global
    rel err, well under the 2e-2 gate.
  - scores matmul: stationary w [128, E] per k-chunk, moving xh [128, 512],
    16 chunks accumulate into one [64, 512] PSUM tile per token group.
  - psum -> SBUF copy on the scalar engine gives sneg = -scores expert-major;
    a pure tensor-engine transpose (plain identity) makes it token-major.
  - softmax skips the max-subtraction (|scores| < ~4: exp safe in fp32; the
    renorm cancels any shift exactly) and drops the +eps term (ws >= 44*e^-4,
    eps*S is ~1e-6 relative).
  - per-subtile: DVE does only max8 x3 + match_replace x2 + select-STT +
    reciprocal; exp and the psum->SBUF copies run on scalar; final
    wm*(1/ws) scale rides a scalar activation-Copy; output fp16.
  - kernel semaphore range shrunk (BOLTZ_SEM_TOP): the fixed preamble/exit
    sem-range clears cost ~30ns/sem; the stock range clears 106.
"""

import os
import sys

sys.path.insert(0, "/opt/trn_rl_repo")

import numpy as np

D = 2048
E = 64
N_BOTTOM = 20  # 64 experts - 44 active
NEG_BIG = -1e30
TEMPERATURE = 2.718281828459045
N_CORES = 8
TPC = 2048  # tokens per core
GROUP = 512  # tokens per matmul group (one PSUM bank)
KC_N = D // 128  # 16 contraction chunks
QN = 4  # kc chunks per x DMA (4 DMAs per group)

_SEM_TOP = int(os.environ.get("BOLTZ_SEM_TOP", "200"))


def _build_nc():
    import concourse.bacc as bacc
    import concourse.mybir as mybir
    from concourse import bass as _bass
    from concourse.masks import make_identity
    from concourse.tile import TileContext

    F32 = mybir.dt.float32
    F16 = mybir.dt.float16
    n_groups = TPC // GROUP

    if _SEM_TOP:
        # the kernel preamble range-clears (and the exit drain waits) every
        # sem in this range at ~30ns each; tile recycles aggressively so a
        # much smaller pool suffices (allocation failure is a loud build
        # error, not a runtime hazard)
        _bass.get_kernel_semaphore_range = lambda: range(
            _bass.get_walrus_max_sem_num(), _SEM_TOP
        )

    if os.environ.get("BOLTZ_LEAN_TAIL", "1") == "1":
        # the stock Tile exit emits drain + barrier + sem-clear + barrier;
        # the kernel preamble already range-clears the semaphores at the
        # start of every execution, so drain + one barrier suffices
        def _lean_drain_and_barrier(self, tick_clock, wait_clock):
            from concourse.tile import ScopedClock

            drain_inst = self.nc.sync.drain()
            wait_clock.add_sem_waits(
                drain_inst.ins, ScopedClock({None: tick_clock.global_clock})
            )
            self.nc.all_engine_barrier()
            popped = self.nc._tile_sem_poison_stack.pop()
            assert popped is self._sem_poison
            self.sems.allocated()

        TileContext._drain_and_barrier = _lean_drain_and_barrier

    nc = bacc.Bacc(None, target_bir_lowering=False)
    xT_d = nc.declare_dram_parameter("xT", [D, TPC], F16, isOutput=False)
    wh_d = nc.declare_dram_parameter("wh", [D, E], F16, isOutput=False)
    out_d = nc.declare_dram_parameter("out", [TPC, E], F16, isOutput=True)

    with TileContext(nc) as tc:
        with (
            tc.tile_pool(name="const", bufs=1) as cpool,
            tc.tile_pool(name="xg", bufs=n_groups) as xpool,
            tc.tile_pool(name="sneg", bufs=2) as snpool,
            tc.tile_pool(name="og", bufs=n_groups) as opool,
            tc.tile_pool(name="work", bufs=3) as wkpool,
            tc.tile_pool(name="small", bufs=8) as smpool,
            tc.tile_pool(name="ps_s", bufs=2, space="PSUM") as pspool,
            tc.tile_pool(name="ps_t", bufs=4, space="PSUM") as ps_t_pool,
        ):
            ident = cpool.tile([E, E], F32)
            make_identity(nc, ident)

            wh_sb = cpool.tile([128, KC_N, E], F16)
            nc.sync.dma_start(
                out=wh_sb, in_=wh_d[:, :].rearrange("(kc p) e -> p kc e", p=128)
            )

            # all x DMAs up front: no dependencies, SP issues them
            # back-to-back so the transfer stream never starves
            xgs = []  # xgs[g][q] = [128, QN, GROUP] fp16
            for g in range(n_groups):
                tiles = []
                for q in range(KC_N // QN):
                    xq = xpool.tile([128, QN, GROUP], F16, tag=f"xq{q}")
                    nc.sync.dma_start(
                        out=xq,
                        in_=xT_d[
                            q * QN * 128 : (q + 1) * QN * 128,
                            g * GROUP : (g + 1) * GROUP,
                        ].rearrange("(c p) t -> p c t", p=128),
                    )
                    tiles.append(xq)
                xgs.append(tiles)

            for g in range(n_groups):
                last = g == n_groups - 1
                og = opool.tile([128, GROUP // 128, E], F16, tag="og")
                ps = pspool.tile([E, GROUP], F32, tag="ps")
                for kc in range(KC_N):
                    nc.tensor.matmul(
                        ps,
                        lhsT=wh_sb[:, kc, :],
                        rhs=xgs[g][kc // QN][:, kc % QN, :],
                        start=(kc == 0),
                        stop=(kc == KC_N - 1),
                    )
                # sneg = -scores (w pre-negated on host), expert-major
                sneg = snpool.tile([E, GROUP], F32, tag="sneg")
                nc.scalar.copy(sneg, ps)

                for si in range(GROUP // 128):
                    # token-major negated scores [128 tok, 64 e]
                    psum_t = ps_t_pool.tile([128, E], F32, tag="ps_t")
                    nc.tensor.transpose(
                        psum_t, sneg[:, si * 128 : (si + 1) * 128], ident
                    )
                    # u = exp(scores) (no max-sub needed: |scores| < ~4)
                    u = wkpool.tile([128, E], F32, tag="u")
                    nc.scalar.activation(
                        u, psum_t, mybir.ActivationFunctionType.Exp, scale=-1.0
                    )
                    # SBUF copy of -scores for the DVE selection chain
                    s_sb = wkpool.tile([128, E], F32, tag="s_sb")
                    nc.scalar.copy(s_sb, psum_t)

                    # threshold = 21st smallest score = 21st largest of
                    # -scores: top-8 rounds with match_replace, then
                    # rank 17-24 -> index 4
                    r1 = smpool.tile([128, 8], F32, tag="r1")
                    nc.vector.max(r1, s_sb)
                    y = wkpool.tile([128, E], F32, tag="y")
                    nc.vector.match_replace(y, r1, s_sb, NEG_BIG)
                    r2 = smpool.tile([128, 8], F32, tag="r2")
                    nc.vector.max(r2, y)
                    nc.vector.match_replace(y, r2, y, NEG_BIG)
                    r3 = smpool.tile([128, 8], F32, tag="r3")
                    nc.vector.max(r3, y)
                    thr = r3[:, (N_BOTTOM - 16) : (N_BOTTOM - 16 + 1)]

                    # wm = u * (-scores <= thr); ws = sum(wm)
                    wm = wkpool.tile([128, E], F32, tag="wm")
                    ws = smpool.tile([128, 1], F32, tag="ws")
                    nc.vector.scalar_tensor_tensor(
                        out=wm,
                        in0=s_sb,
                        scalar=thr,
                        in1=u,
                        op0=mybir.AluOpType.is_le,
                        op1=mybir.AluOpType.mult,
                        accum_out=ws,
                    )
                    # out = wm / ws (the +eps term is ~1e-6 relative: dropped)
                    rd = smpool.tile([128, 1], F32, tag="rd")
                    nc.vector.reciprocal(rd, ws)
                    nc.scalar.mul(og[:, si, :], wm, rd)

                    if last:
                        # per-subtile drain on the final group: the very last
                        # output DMA then carries only one subtile's 16KB
                        nc.sync.dma_start(
                            out=out_d[
                                g * GROUP + si * 128 : g * GROUP + (si + 1) * 128, :
                            ],
                            in_=og[:, si, :],
                        )
                if not last:
                    # inline output DMA: all x DMAs are already issued, so
                    # this never delays a prefetch
                    nc.sync.dma_start(
                        out=out_d[g * GROUP : (g + 1) * GROUP, :].rearrange(
                            "(s p) e -> p s e", p=128
                        ),
                        in_=og,
                    )

    nc.finalize()
    return nc


_NC = None
LAST_EXEC_NS = None
LAST_RESULTS = None


def _get_nc():
    global _NC
    if _NC is None:
        _NC = _build_nc()
    return _NC


def kernel(x, gate_w, trace=False):
    global LAST_EXEC_NS, LAST_RESULTS
    from concourse.bass_utils import run_bass_kernel_spmd

    x = np.asarray(x)
    gate_w = np.asarray(gate_w)
    Btot = x.shape[0] * x.shape[1]
    x2 = x.reshape(Btot, D).astype(np.float32, copy=False)
    # negated so the device PSUM holds -scores directly
    wh = (-gate_w.astype(np.float32).T / np.float32(TEMPERATURE)).astype(np.float16)
    wh = np.ascontiguousarray(wh)

    nc = _get_nc()
    in_maps = []
    for i in range(N_CORES):
        shard = np.ascontiguousarray(x2[i * TPC : (i + 1) * TPC].T.astype(np.float16))
        in_maps.append({"xT": shard, "wh": wh})

    kwargs = {}
    if trace:
        try:
            import antenv.axon_hooks  # noqa: F401  (registered by tracehook)

            kwargs["trace"] = True
        except ImportError:
            pass
    res = run_bass_kernel_spmd(nc, in_maps, core_ids=list(range(N_CORES)), **kwargs)
    LAST_EXEC_NS = res.exec_time_ns
    LAST_RESULTS = res
    out = np.concatenate([res.results[i]["out"] for i in range(N_CORES)], axis=0)
    return out.reshape(x.shape[0], x.shape[1], E).astype(np.float32)
